# revision 10
# baseline (speedup 1.0000x reference)
"""Trainium2 Bass kernel for CantorGlobalAttention (sparse routed attention).

Strategy: the routes table is shared across batch and heads, so the sparse
gather-attention is reformulated as dense attention using a host-precomputed
route-multiplicity matrix m[s,j] = #{k: routes[s,k] = j}:

    out[s] = (sum_j m[s,j] exp(SC[s,j]) v[j]) / (sum_j m[s,j] exp(SC[s,j]))
    SC = q @ k^T / sqrt(HD)

Everything runs in a transposed layout (feature dim on partitions) so no
on-device transposes are needed anywhere:
    qkT[n,s]  = (W_qk^T x^T)              (W stationary)
    SCT[j,s]  = k^T(j-tile)^T q^T         (K=64 matmul)
    ET        = mT * exp(0.125 * SCT)     (ACT exp + DVE mult, bf16)
    o2T[c,s]  = [v|1]^T @ ET              (ones col -> softmax denom Z)
    outT      = o2T[0:64] * (1/Z)         (Z bcast on GPSIMD, recip on DVE)
    y[s,n]    = outT^T @ W_proj(rows)     (per-core partial, bf16 out)

Phase D runs a lag-1 software pipeline: the EV matmuls for j-tile jt-1 are
emitted after the SCT matmuls for jt, so the PE never waits on the ACT
exp / DVE multiply chain. B (remaining qk proj), C (v proj) and E (output
proj, first s-half) are folded into phase-D windows as PE filler.

Sharding: 8 cores = 2 batches x 4 head-groups (4 heads each). Host sums the
4 per-batch partials and adds b_proj.
"""

import numpy as np
import ml_dtypes
from contextlib import ExitStack

import concourse.bacc as bacc
import concourse.mybir as mybir
import concourse.tile as tile
from concourse.bass import ts
from concourse.bass_utils import run_bass_kernel_spmd

bf16 = ml_dtypes.bfloat16
F32 = mybir.dt.float32
BF16 = mybir.dt.bfloat16
Alu = mybir.AluOpType
Act = mybir.ActivationFunctionType

B, S, D = 2, 2048, 1024
H, HD, K = 16, 64, 64
NCORES = 8
HG = 4            # head-groups (cores per batch)
NH = H // HG      # heads per core = 4
DH = NH * HD      # feature cols per core for q/k/v = 256
ST = S // 128     # 16 s-tiles
JT = S // 128     # 16 j-tiles
KT = D // 128     # 8 contraction tiles for the projections
SCK = 1024        # s-chunk for phase E / o2s staging
NSC = S // SCK    # 2

_CACHED_NC = None
_LAST_RESULTS = None


def _build_bass():
    nc = bacc.Bacc("TRN2", target_bir_lowering=False, debug=False)

    xT_d = nc.dram_tensor("xT", [128, KT, S], BF16, kind="ExternalInput")
    wqk_d = nc.dram_tensor("wqk", [128, KT, 3 * DH], BF16, kind="ExternalInput")
    wproj_d = nc.dram_tensor("wproj", [128, 2, D], BF16, kind="ExternalInput")
    mt_d = nc.dram_tensor("mt", [128, NSC, JT, SCK], BF16, kind="ExternalInput")
    bqk_d = nc.dram_tensor("bqk", [128, 6], F32, kind="ExternalInput")
    y_d = nc.dram_tensor("y", [KT, 128, S], BF16, kind="ExternalOutput")

    with tile.TileContext(nc) as tc, ExitStack() as ctx:
        cp = ctx.enter_context(tc.tile_pool(name="consts", bufs=1))

        wqk_sb = cp.tile([128, KT, 3 * DH], BF16)
        wproj_sb = cp.tile([128, 2, D], BF16)
        mt_sb = cp.tile([128, NSC, JT, SCK], BF16)
        bqk_sb = cp.tile([128, 6], F32)
        ones_f32 = cp.tile([128, 128], F32)
        qkT_sb = cp.tile([128, 4, S], BF16)  # nt 0,1 = qT; 2,3 = kT
        vext_sb = cp.tile([128, ST, NH, HD + 1], BF16)
        outT_sb = cp.tile([128, 2, S], BF16)     # proj lhsT layout

        xtp = tc.alloc_tile_pool(name="xtp", bufs=1)
        xT_sb = xtp.tile([128, KT, S], BF16)

        # loads, in first-use order; wqk/xT interleaved per kt so the first
        # phase-B wave starts as early as possible; mt comes as scH halves
        # so phase D can start as soon as the first scH0 j-tiles land
        for kt in range(KT):
            nc.sync.dma_start(wqk_sb[:, kt, :], wqk_d[:, kt, :])
            nc.sync.dma_start(xT_sb[:, kt, :], xT_d[:, kt, :])
        nc.sync.dma_start(bqk_sb[:], bqk_d[:])
        for jt in range(JT):
            nc.sync.dma_start(mt_sb[:, 0, jt, :], mt_d[:, 0, jt, :])
        nc.sync.dma_start(wproj_sb[:], wproj_d[:])
        for jt in range(JT):
            nc.sync.dma_start(mt_sb[:, 1, jt, :], mt_d[:, 1, jt, :])

        nc.vector.memset(ones_f32[:], 1.0)
        # ones column FIRST so the softmax denom Z lands on partition 0 of
        # po2/o2s (gpsimd partition_broadcast can only source partition 0)
        nc.vector.memset(vext_sb[:, :, :, 0:1], 1.0)
        # touch Exp now so the ACT table set loads during the DMA window,
        # not at the first real exp
        tblwarm = cp.tile([1, 8], F32)
        nc.scalar.activation(tblwarm[:], ones_f32[0:1, 0:8], Act.Exp)

        # ---- PE warmup during the initial DMA window (HAM clock gate) ----
        pwarm = tc.alloc_tile_pool(name="pwarm", bufs=1, space="PSUM")
        warm = pwarm.tile([128, 128], F32, tag="warm", bufs=1)
        for _ in range(24):
            nc.tensor.matmul(warm[:], ones_f32[:], ones_f32[:],
                             start=True, stop=True, skip_group_check=True)
        pwarm.release()

        def emit_copy(nt, sq, pq, eng):
            if eng == 0:
                nc.vector.tensor_scalar(
                    qkT_sb[:, nt, ts(sq, 512)], pq[:],
                    bqk_sb[:, nt:nt + 1], None, Alu.add)
            else:
                nc.scalar.activation(
                    qkT_sb[:, nt, ts(sq, 512)], pq[:],
                    Act.Identity, bias=bqk_sb[:, nt:nt + 1])

        # ---- phase B (q,k of pair 0) as kt-waves: matmuls start as each xT
        # k-tile DMA lands; one LDWEIGHTS covers four matmuls ----
        pbp = tc.alloc_tile_pool(name="pbp", bufs=1, space="PSUM")
        b02 = [(nt, sq) for nt in (0, 2) for sq in range(S // 512)]
        pqs = {g: pbp.tile([128, 512], F32, tag=f"pq_{g[0]}_{g[1]}",
                           name=f"pq_{g[0]}_{g[1]}") for g in b02}
        for kt in range(KT):
            for nt in (0, 2):
                for sq in range(S // 512):
                    nc.tensor.matmul(
                        pqs[(nt, sq)][:],
                        wqk_sb[:, kt, ts(nt, 128)],
                        xT_sb[:, kt, ts(sq, 512)],
                        start=(kt == 0), stop=(kt == KT - 1),
                    )
        for sq in range(S // 512):           # sq0 copies first: D needs them
            for i, nt in enumerate((0, 2)):
                eng = 0 if sq == 0 else (sq + i) % 2
                emit_copy(nt, sq, pqs[(nt, sq)], eng)
        pbp.release()

        def emit_c(st, pool):
            """v[st] in normal layout (x-stationary) -> vext; copy on DVE/ACT."""
            pv = pool.tile([128, DH], F32, tag="pv")
            for kt in range(KT):
                nc.tensor.matmul(
                    pv[:], xT_sb[:, kt, ts(st, 128)],
                    wqk_sb[:, kt, 4 * 128:6 * 128],
                    start=(kt == 0), stop=(kt == KT - 1))
            dst = vext_sb[:, st, :, 1:HD + 1]
            src = pv[:].rearrange("p (h d) -> p h d", d=HD)
            nc.vector.tensor_copy(dst, src)

        def emit_b_pair(nt, sp, pool):
            # two sq chunks, kt-major: one LDWEIGHTS serves two matmuls
            sqs = (2 * sp, 2 * sp + 1)
            pq2 = {sq: pool.tile([128, 512], F32, tag=f"pq2_{sq % 2}",
                                 name=f"pq2_{nt}_{sq}") for sq in sqs}
            for kt in range(KT):
                for sq in sqs:
                    nc.tensor.matmul(
                        pq2[sq][:],
                        wqk_sb[:, kt, ts(nt, 128)],
                        xT_sb[:, kt, ts(sq, 512)],
                        start=(kt == 0), stop=(kt == KT - 1),
                    )
            for i, sq in enumerate(sqs):
                emit_copy(nt, sq, pq2[sq], i)

        # ---- phase D: attention, lag-1 pipelined ----
        dwork = tc.alloc_tile_pool(name="dwork", bufs=4)
        o2pool = tc.alloc_tile_pool(name="o2pool", bufs=1)
        nwork = tc.alloc_tile_pool(name="nwork", bufs=2)

        pscp = tc.alloc_tile_pool(name="pscp", bufs=2, space="PSUM")
        po2p = tc.alloc_tile_pool(name="po2p", bufs=1, space="PSUM")
        aux = tc.alloc_tile_pool(name="pvp", bufs=2, space="PSUM")

        ystage = tc.alloc_tile_pool(name="ystage", bufs=3)

        norm_state = {}

        def norm_start(o2s_t, hl, scH):
            # Z (partition 0) broadcast on GPSIMD; recip/scale follow as
            # half-width DVE pieces spread over later slots
            zb = nwork.tile([HD + 1, SCK], F32, tag="zb", bufs=2)
            nc.gpsimd.partition_broadcast(zb[:], o2s_t[0:1, :])
            rz = nwork.tile([HD + 1, SCK], F32, tag="rz", bufs=2)
            stage = nwork.tile([HD + 1, SCK], BF16, tag="stage", bufs=2)
            norm_state[(hl, scH)] = (o2s_t, zb, rz, stage)

        def norm_piece(hl, scH, half):
            o2s_t, zb, rz, stage = norm_state[(hl, scH)]
            sl = ts(half, 512)
            nc.vector.reciprocal_approx_fast(out=rz[:, sl], in_=zb[:, sl])
            nc.gpsimd.tensor_tensor(stage[:, sl], o2s_t[:, sl], rz[:, sl],
                                    Alu.mult)
            if half == 1:
                p0 = (hl % 2) * 64
                nc.sync.dma_start(
                    outT_sb[p0:p0 + 64, hl // 2, ts(scH, SCK)],
                    stage[1:HD + 1, :])

        bstate = {}

        def b_chunk(nt, sq, part, pool):
            # half of a (nt, sq) qk-projection chunk: 4 kt steps; the second
            # half finishes the accumulation and drains with the bias add
            if part == 0:
                bstate[(nt, sq)] = pool.tile([128, 512], F32, tag="pq",
                                             name=f"pq_{nt}_{sq}")
            pq = bstate[(nt, sq)]
            for kt in range(part * 4, part * 4 + 4):
                nc.tensor.matmul(pq[:], wqk_sb[:, kt, ts(nt, 128)],
                                 xT_sb[:, kt, ts(sq, 512)],
                                 start=(kt == 0), stop=(kt == KT - 1))
            if part == 1:
                nc.vector.tensor_scalar(qkT_sb[:, nt, ts(sq, 512)], pq[:],
                                        bqk_sb[:, nt:nt + 1], None, Alu.add)

        def emit_proj(nt, sh, pool, cast, tag="py"):
            # weight-stationary: yT[n, s] = sum_i wproj[i, n] outT[i, s]
            py = pool.tile([128, SCK], F32, tag=tag)
            for it in range(2):
                for sq in range(2):
                    nc.tensor.matmul(
                        py[:, ts(sq, 512)],
                        wproj_sb[:, it, ts(nt, 128)],
                        outT_sb[:, it, sh * SCK + sq * 512:
                                sh * SCK + sq * 512 + 512],
                        start=(it == 0), stop=(it == 1))
            y_sb = ystage.tile([128, SCK], BF16, tag="y_sb")
            if cast == "dve":
                nc.vector.tensor_copy(y_sb[:], py[:])
            elif cast == "act":
                nc.scalar.copy(y_sb[:], py[:])
            else:
                nc.vector.tensor_copy(y_sb[:, 0:512], py[:, 0:512])
                nc.scalar.copy(y_sb[:, 512:SCK], py[:, 512:SCK])
            nc.sync.dma_start(y_d[nt, :, ts(sh, SCK)], y_sb[:])

        o2s_all = {}
        for pair in range(2):
            ha, hb = 2 * pair, 2 * pair + 1
            qT2 = qkT_sb[:, pair, :]
            kT2 = qkT_sb[:, 2 + pair, :]
            o2s = {}
            for hl in (ha, hb):
                for scH in range(NSC):
                    o2s[(hl, scH)] = o2pool.tile(
                        [HD + 1, SCK], F32, tag=f"o2s_{hl % 2}_{scH}",
                        name=f"o2s_{hl}_{scH}")
            o2s_all.update(o2s)
            for sc4 in range(4):                  # s chunks of 512
                scH, half = sc4 // 2, sc4 % 2
                s0 = sc4 * 512

                po2_a = po2p.tile([HD + 1, 512], F32, tag="po2a")
                po2_b = po2p.tile([HD + 1, 512], F32, tag="po2b")

                def emit_ev(jt, et, stop):
                    nc.tensor.matmul(po2_a[:], vext_sb[:, jt, ha, :],
                                     et[:, 0:512],
                                     start=(jt == 0), stop=stop)
                    nc.tensor.matmul(po2_b[:], vext_sb[:, jt, hb, :],
                                     et[:, 512:1024],
                                     start=(jt == 0), stop=stop)

                def filler(jt):
                    w = (pair, sc4)
                    if w == (0, 0):
                        emit_c(jt, aux)
                    elif w in ((0, 1), (0, 2), (0, 3), (1, 0), (1, 1)):
                        # spread qk-projection remainder; per-window norm
                        chunks = {(0, 1): [(3, 0), (3, 1)],
                                  (0, 2): [(3, 2), (3, 3)],
                                  (0, 3): [(1, 0), (1, 1)],
                                  (1, 0): [(1, 2)],
                                  (1, 1): [(1, 3)]}[w]
                        if jt in (0, 4, 8, 12) and jt // 8 < len(chunks):
                            nt, sq = chunks[jt // 8]
                            b_chunk(nt, sq, (jt // 4) % 2, aux)
                        nrm = {(0, 2): (0, 0), (0, 3): (1, 0),
                               (1, 0): (0, 1), (1, 1): (1, 1)}.get(w)
                        if nrm is not None:
                            if jt == 2:
                                norm_start(o2s_all[nrm], *nrm)
                            elif jt == 3:
                                norm_piece(*nrm, 0)
                            elif jt == 5:
                                norm_piece(*nrm, 1)
                    elif w == (1, 2):
                        if jt == 0:
                            norm_start(o2s[(2, 0)], 2, 0)
                        elif jt == 1:
                            norm_piece(2, 0, 0)
                        elif jt == 2:
                            norm_piece(2, 0, 1)
                        elif jt == 3:
                            norm_start(o2s[(3, 0)], 3, 0)
                        elif jt == 4:
                            norm_piece(3, 0, 0)
                        elif jt == 5:
                            norm_piece(3, 0, 1)
                        elif jt == 9:
                            emit_proj(0, 0, aux, cast="act")
                        elif jt == 12:
                            emit_proj(1, 0, aux, cast="act")
                    elif w == (1, 3):
                        if jt in (1, 4, 7, 10):
                            emit_proj(2 + (jt - 1) // 3, 0, aux,
                                      cast=("act" if jt < 6 else "split"))

                pend = []           # lag-2 EV pipeline
                for jt in range(JT):
                    psc = pscp.tile([128, SCK], F32, tag="psc")
                    nc.tensor.matmul(
                        psc[:, 0:512], kT2[0:64, ts(jt, 128)],
                        qT2[0:64, s0:s0 + 512], start=True, stop=True)
                    nc.tensor.matmul(
                        psc[:, 512:1024], kT2[64:128, ts(jt, 128)],
                        qT2[64:128, s0:s0 + 512], start=True, stop=True)
                    et = dwork.tile([128, SCK], BF16, tag="et")
                    nc.scalar.activation(et[:], psc[:], Act.Exp, scale=0.125)
                    mtsl = mt_sb[:, scH, jt, ts(half, 512)]
                    # every 4th multiply runs on the otherwise-idle GPSIMD
                    eng = nc.gpsimd if jt % 4 == 2 else nc.vector
                    eng.tensor_tensor(et[:, 0:512], et[:, 0:512],
                                      mtsl, Alu.mult)
                    eng.tensor_tensor(et[:, 512:1024], et[:, 512:1024],
                                      mtsl, Alu.mult)
                    pend.append((jt, et))
                    if len(pend) > 2:
                        pj, pet = pend.pop(0)
                        emit_ev(pj, pet, stop=False)
                    filler(jt)

                for pj, pet in pend:
                    emit_ev(pj, pet, stop=(pj == JT - 1))
                # drain po2 -> o2s staging (DVE; ACT keeps only the exps)
                nc.vector.tensor_copy(o2s[(ha, scH)][:, ts(half, 512)],
                                      po2_a[:])
                nc.vector.tensor_copy(o2s[(hb, scH)][:, ts(half, 512)],
                                      po2_b[:])

                if pair == 0 and sc4 == 0:
                    # v done: aux becomes the phase-B remainder pool
                    aux.release()
                    aux = tc.alloc_tile_pool(name="pb2", bufs=2, space="PSUM")
                elif pair == 1 and sc4 == 1:
                    # B done: aux becomes the phase-E pool (1 buf: 2 banks)
                    aux.release()
                    aux = tc.alloc_tile_pool(name="pe0", bufs=1, space="PSUM")

        # ---- tail: leftover sh0 projections first (their casts land at
        # the head of the DVE queue so the py-tag WARs resolve fast), then
        # the final norms, then the sh1 projections staggered ----
        aux.release()
        po2p.release()
        pscp.release()
        pep = tc.alloc_tile_pool(name="pe1", bufs=1, space="PSUM")
        wt = tc.alloc_tile_pool(name="wt", bufs=1, space="PSUM")
        warm2 = wt.tile([128, 128], F32, tag="w2", bufs=1)

        def wspace(n):
            for _ in range(n):
                nc.tensor.matmul(warm2[:], ones_f32[:], ones_f32[:],
                                 start=True, stop=True,
                                 skip_group_check=True)

        emit_proj(6, 0, pep, cast="dve", tag="py1_0")
        wspace(1)
        emit_proj(7, 0, pep, cast="act", tag="py1_1")
        norm_start(o2s_all[(2, 1)], 2, 1)
        norm_piece(2, 1, 0)
        norm_piece(2, 1, 1)
        norm_start(o2s_all[(3, 1)], 3, 1)
        norm_piece(3, 1, 0)
        norm_piece(3, 1, 1)
        wspace(2)

        pys = {}
        for nt in range(KT):
            py = pep.tile([128, SCK], F32, tag=f"py1_{nt % 3}",
                          name=f"py1_{nt}")
            pys[nt] = py
            nc.tensor.matmul(py[:, 0:512], wproj_sb[:, 0, ts(nt, 128)],
                             outT_sb[:, 0, SCK:SCK + 512],
                             start=True, stop=False)
            nc.tensor.matmul(py[:, 512:SCK], wproj_sb[:, 0, ts(nt, 128)],
                             outT_sb[:, 0, SCK + 512:2 * SCK],
                             start=True, stop=False)
            if nt < 5:
                wspace(2)
            if nt >= 2:
                finish_nt = nt - 2
                pyf = pys[finish_nt]
                for sq in range(2):
                    nc.tensor.matmul(
                        pyf[:, ts(sq, 512)],
                        wproj_sb[:, 1, ts(finish_nt, 128)],
                        outT_sb[:, 1, SCK + sq * 512:SCK + sq * 512 + 512],
                        start=False, stop=True)
                y_sb = ystage.tile([128, SCK], BF16, tag="y_sb")
                nc.vector.tensor_copy(y_sb[:, 0:512], pyf[:, 0:512])
                nc.scalar.copy(y_sb[:, 512:SCK], pyf[:, 512:SCK])
                nc.sync.dma_start(y_d[finish_nt, :, ts(1, SCK)], y_sb[:])
        for nt in range(KT - 2, KT):
            pyf = pys[nt]
            for sq in range(2):
                nc.tensor.matmul(
                    pyf[:, ts(sq, 512)],
                    wproj_sb[:, 1, ts(nt, 128)],
                    outT_sb[:, 1, SCK + sq * 512:SCK + sq * 512 + 512],
                    start=False, stop=True)
            y_sb = ystage.tile([128, SCK], BF16, tag="y_sb")
            nc.vector.tensor_copy(y_sb[:, 0:512], pyf[:, 0:512])
            nc.scalar.copy(y_sb[:, 512:SCK], pyf[:, 512:SCK])
            nc.sync.dma_start(y_d[nt, :, ts(1, SCK)], y_sb[:])

        wt.release()
        pep.release()
        ystage.release()
        nwork.release()
        o2pool.release()
        dwork.release()
        xtp.release()

    nc.compile()
    return nc


def _get_nc():
    global _CACHED_NC
    if _CACHED_NC is None:
        _CACHED_NC = _build_bass()
    return _CACHED_NC


def _prep_core_inputs(x, W_qkv, b_qkv, W_proj, routes_m_T):
    """Host-side shard prep for one (batch b, head-group hg) core."""
    maps = []
    for core in range(NCORES):
        b, hg = core // HG, core % HG
        c0 = hg * DH
        xT = np.ascontiguousarray(x[b].T).astype(bf16)            # (1024, 2048)
        wqk = np.concatenate(
            [W_qkv[:, c0:c0 + DH], W_qkv[:, D + c0:D + c0 + DH],
             W_qkv[:, 2 * D + c0:2 * D + c0 + DH]], axis=1)        # (1024, 768)
        bqk = np.concatenate([b_qkv[c0:c0 + DH], b_qkv[D + c0:D + c0 + DH],
                              b_qkv[2 * D + c0:2 * D + c0 + DH]])
        wproj = W_proj[c0:c0 + DH, :]                              # (256, 1024)
        maps.append({
            "xT": np.ascontiguousarray(xT.reshape(KT, 128, S).transpose(1, 0, 2)),
            "wqk": np.ascontiguousarray(
                wqk.astype(bf16).reshape(KT, 128, 3 * DH).transpose(1, 0, 2)),
            "wproj": np.ascontiguousarray(
                wproj.astype(bf16).reshape(2, 128, D).transpose(1, 0, 2)),
            "mt": routes_m_T,
            "bqk": np.ascontiguousarray(
                bqk.astype(np.float32).reshape(6, 128).T),
        })
    return maps


def kernel(x, W_qkv, b_qkv, W_proj, b_proj, routes):
    x = np.asarray(x, dtype=np.float32)
    W_qkv = np.asarray(W_qkv, dtype=np.float32)
    b_qkv = np.asarray(b_qkv, dtype=np.float32)
    W_proj = np.asarray(W_proj, dtype=np.float32)
    b_proj = np.asarray(b_proj, dtype=np.float32)
    r = np.clip(np.asarray(routes).astype(np.int64), 0, S - 1)

    # multiplicity matrix, uploaded transposed in scH halves:
    # mt[p, scH, jt, sH] = m[scH*1024 + sH, jt*128 + p]
    m = np.zeros((S, S), dtype=np.float32)
    np.add.at(m, (np.arange(S)[:, None].repeat(K, 1).ravel(), r.ravel()), 1.0)
    mT = np.ascontiguousarray(
        m.T.astype(bf16).reshape(JT, 128, NSC, SCK).transpose(1, 2, 0, 3))

    nc = _get_nc()
    in_maps = _prep_core_inputs(x, W_qkv, b_qkv, W_proj, mT)
    res = run_bass_kernel_spmd(nc, in_maps, core_ids=list(range(NCORES)))
    global _LAST_RESULTS
    _LAST_RESULTS = res

    y = np.zeros((B, S, D), dtype=np.float32)
    for core in range(NCORES):
        b = core // HG
        yT = res.results[core]["y"].astype(np.float32)   # (KT, 128, S)
        y[b] += yT.reshape(D, S).T
    y += b_proj[None, None, :]
    return y


# revision 11
# speedup vs baseline: 1.2772x; 1.2772x over previous
"""Trainium2 Bass kernel for CantorGlobalAttention (sparse routed attention).

Strategy: the routes table is shared across batch and heads, so the sparse
gather-attention is reformulated as dense attention using a host-precomputed
route-multiplicity matrix m[s,j] = #{k: routes[s,k] = j}:

    out[s] = (sum_j m[s,j] exp(SC[s,j]) v[j]) / (sum_j m[s,j] exp(SC[s,j]))
    SC = q @ k^T / sqrt(HD)

Everything runs in a transposed layout (feature dim on partitions) so no
on-device transposes are needed anywhere:
    qkT[n,s]  = (W_qk^T x^T)              (W stationary)
    SCT[j,s]  = k^T(j-tile)^T q^T         (K=64 matmul)
    ET        = mT * exp(0.125 * SCT)     (ACT exp + DVE mult, bf16)
    o2T[c,s]  = [v|1]^T @ ET              (ones col -> softmax denom Z)
    outT      = o2T[0:64] * (1/Z)         (Z bcast on GPSIMD, recip on DVE)
    y[s,n]    = outT^T @ W_proj(rows)     (per-core partial, bf16 out)

Phase D runs a lag-1 software pipeline: the EV matmuls for j-tile jt-1 are
emitted after the SCT matmuls for jt, so the PE never waits on the ACT
exp / DVE multiply chain. B (remaining qk proj), C (v proj) and E (output
proj, first s-half) are folded into phase-D windows as PE filler.

Sharding: 8 cores = 2 batches x 4 head-groups (4 heads each). Host sums the
4 per-batch partials and adds b_proj.
"""

import numpy as np
import ml_dtypes
from contextlib import ExitStack

import concourse.bacc as bacc
import concourse.mybir as mybir
import concourse.tile as tile
from concourse.bass import ts
from concourse.bass_utils import run_bass_kernel_spmd

bf16 = ml_dtypes.bfloat16
F32 = mybir.dt.float32
BF16 = mybir.dt.bfloat16
Alu = mybir.AluOpType
Act = mybir.ActivationFunctionType

B, S, D = 2, 2048, 1024
H, HD, K = 16, 64, 64
NCORES = 8
HG = 4            # head-groups (cores per batch)
NH = H // HG      # heads per core = 4
DH = NH * HD      # feature cols per core for q/k/v = 256
ST = S // 128     # 16 s-tiles
JT = S // 128     # 16 j-tiles
KT = D // 128     # 8 contraction tiles for the projections
SCK = 1024        # s-chunk for phase E / o2s staging
NSC = S // SCK    # 2

_CACHED_NC = None
_LAST_RESULTS = None


def _build_bass():
    nc = bacc.Bacc("TRN2", target_bir_lowering=False, debug=False)

    xT_d = nc.dram_tensor("xT", [128, KT, S], BF16, kind="ExternalInput")
    wqk_d = nc.dram_tensor("wqk", [128, KT, 3 * DH], BF16, kind="ExternalInput")
    wproj_d = nc.dram_tensor("wproj", [128, 2, D], BF16, kind="ExternalInput")
    mt_d = nc.dram_tensor("mt", [128, NSC, JT, SCK], BF16, kind="ExternalInput")
    bqk_d = nc.dram_tensor("bqk", [128, 6], F32, kind="ExternalInput")
    y_d = nc.dram_tensor("y", [KT, 128, S], BF16, kind="ExternalOutput")

    with tile.TileContext(nc) as tc, ExitStack() as ctx:
        cp = ctx.enter_context(tc.tile_pool(name="consts", bufs=1))

        wqk_sb = cp.tile([128, KT, 3 * DH], BF16)
        wproj_sb = cp.tile([128, 2, D], BF16)
        mt_sb = cp.tile([128, NSC, JT, SCK], BF16)
        bqk_sb = cp.tile([128, 6], F32)
        ones_f32 = cp.tile([128, 128], F32)
        qkT_sb = cp.tile([128, 4, S], BF16)  # nt 0,1 = qT; 2,3 = kT
        vext_sb = cp.tile([128, ST, NH, HD + 1], BF16)
        outT_sb = cp.tile([128, 2, S], BF16)     # proj lhsT layout

        xtp = tc.alloc_tile_pool(name="xtp", bufs=1)
        xT_sb = xtp.tile([128, KT, S], BF16)

        # loads, in first-use order; wqk/xT interleaved per kt so the first
        # phase-B wave starts as early as possible; mt comes as scH halves
        # so phase D can start as soon as the first scH0 j-tiles land
        for kt in range(KT):
            nc.sync.dma_start(wqk_sb[:, kt, :], wqk_d[:, kt, :])
            nc.sync.dma_start(xT_sb[:, kt, :], xT_d[:, kt, :])
        nc.sync.dma_start(bqk_sb[:], bqk_d[:])
        for jt in range(JT):
            nc.sync.dma_start(mt_sb[:, 0, jt, :], mt_d[:, 0, jt, :])
        nc.sync.dma_start(wproj_sb[:], wproj_d[:])
        for jt in range(JT):
            nc.sync.dma_start(mt_sb[:, 1, jt, :], mt_d[:, 1, jt, :])

        nc.vector.memset(ones_f32[:], 1.0)
        # ones column FIRST so the softmax denom Z lands on partition 0 of
        # po2/o2s (gpsimd partition_broadcast can only source partition 0)
        nc.vector.memset(vext_sb[:, :, :, 0:1], 1.0)
        # touch Exp now so the ACT table set loads during the DMA window,
        # not at the first real exp
        tblwarm = cp.tile([1, 8], F32)
        nc.scalar.activation(tblwarm[:], ones_f32[0:1, 0:8], Act.Exp)

        # ---- PE warmup during the initial DMA window (HAM clock gate) ----
        pwarm = tc.alloc_tile_pool(name="pwarm", bufs=1, space="PSUM")
        warm = pwarm.tile([128, 128], F32, tag="warm", bufs=1)
        for _ in range(24):
            nc.tensor.matmul(warm[:], ones_f32[:], ones_f32[:],
                             start=True, stop=True, skip_group_check=True)
        pwarm.release()

        def emit_copy(nt, sq, pq, eng):
            if eng == 0:
                nc.vector.tensor_scalar(
                    qkT_sb[:, nt, ts(sq, 512)], pq[:],
                    bqk_sb[:, nt:nt + 1], None, Alu.add)
            else:
                nc.scalar.activation(
                    qkT_sb[:, nt, ts(sq, 512)], pq[:],
                    Act.Identity, bias=bqk_sb[:, nt:nt + 1])

        # ---- phase B (q,k of pair 0) as kt-waves: matmuls start as each xT
        # k-tile DMA lands; one LDWEIGHTS covers four matmuls ----
        pbp = tc.alloc_tile_pool(name="pbp", bufs=1, space="PSUM")
        b02 = [(nt, sq) for nt in (0, 2) for sq in range(S // 512)]
        pqs = {g: pbp.tile([128, 512], F32, tag=f"pq_{g[0]}_{g[1]}",
                           name=f"pq_{g[0]}_{g[1]}") for g in b02}
        for kt in range(KT):
            for nt in (0, 2):
                for sq in range(S // 512):
                    nc.tensor.matmul(
                        pqs[(nt, sq)][:],
                        wqk_sb[:, kt, ts(nt, 128)],
                        xT_sb[:, kt, ts(sq, 512)],
                        start=(kt == 0), stop=(kt == KT - 1),
                    )
        for sq in range(S // 512):           # sq0 copies first: D needs them
            for i, nt in enumerate((0, 2)):
                eng = 0 if sq == 0 else (sq + i) % 2
                emit_copy(nt, sq, pqs[(nt, sq)], eng)
        pbp.release()

        def emit_c(st, pool):
            """v[st] in normal layout (x-stationary) -> vext; copy on DVE/ACT."""
            pv = pool.tile([128, DH], F32, tag="pv")
            for kt in range(KT):
                nc.tensor.matmul(
                    pv[:], xT_sb[:, kt, ts(st, 128)],
                    wqk_sb[:, kt, 4 * 128:6 * 128],
                    start=(kt == 0), stop=(kt == KT - 1))
            dst = vext_sb[:, st, :, 1:HD + 1]
            src = pv[:].rearrange("p (h d) -> p h d", d=HD)
            nc.vector.tensor_copy(dst, src)

        def emit_b_pair(nt, sp, pool):
            # two sq chunks, kt-major: one LDWEIGHTS serves two matmuls
            sqs = (2 * sp, 2 * sp + 1)
            pq2 = {sq: pool.tile([128, 512], F32, tag=f"pq2_{sq % 2}",
                                 name=f"pq2_{nt}_{sq}") for sq in sqs}
            for kt in range(KT):
                for sq in sqs:
                    nc.tensor.matmul(
                        pq2[sq][:],
                        wqk_sb[:, kt, ts(nt, 128)],
                        xT_sb[:, kt, ts(sq, 512)],
                        start=(kt == 0), stop=(kt == KT - 1),
                    )
            for i, sq in enumerate(sqs):
                emit_copy(nt, sq, pq2[sq], i)

        # ---- phase D: attention, lag-1 pipelined ----
        dwork = tc.alloc_tile_pool(name="dwork", bufs=4)
        o2pool = tc.alloc_tile_pool(name="o2pool", bufs=1)
        nwork = tc.alloc_tile_pool(name="nwork", bufs=2)

        pscp = tc.alloc_tile_pool(name="pscp", bufs=2, space="PSUM")
        po2p = tc.alloc_tile_pool(name="po2p", bufs=1, space="PSUM")
        aux = tc.alloc_tile_pool(name="pvp", bufs=2, space="PSUM")

        ystage = tc.alloc_tile_pool(name="ystage", bufs=3)

        norm_state = {}

        def norm_start(o2s_t, hl, scH):
            # Z (partition 0) broadcast on GPSIMD; recip/scale follow as
            # half-width DVE pieces spread over later slots
            zb = nwork.tile([HD + 1, SCK], F32, tag="zb", bufs=2)
            nc.gpsimd.partition_broadcast(zb[:], o2s_t[0:1, :])
            rz = nwork.tile([HD + 1, SCK], F32, tag="rz", bufs=2)
            stage = nwork.tile([HD + 1, SCK], BF16, tag="stage", bufs=2)
            norm_state[(hl, scH)] = (o2s_t, zb, rz, stage)

        def norm_piece(hl, scH, half):
            o2s_t, zb, rz, stage = norm_state[(hl, scH)]
            sl = ts(half, 512)
            nc.vector.reciprocal_approx_fast(out=rz[:, sl], in_=zb[:, sl])
            nc.vector.tensor_tensor(stage[:, sl], o2s_t[:, sl], rz[:, sl],
                                    Alu.mult)
            if half == 1:
                p0 = (hl % 2) * 64
                nc.sync.dma_start(
                    outT_sb[p0:p0 + 64, hl // 2, ts(scH, SCK)],
                    stage[1:HD + 1, :])

        bstate = {}

        def b_chunk(nt, sq, part, pool):
            # half of a (nt, sq) qk-projection chunk: 4 kt steps; the second
            # half finishes the accumulation and drains with the bias add
            if part == 0:
                bstate[(nt, sq)] = pool.tile([128, 512], F32, tag="pq",
                                             name=f"pq_{nt}_{sq}")
            pq = bstate[(nt, sq)]
            for kt in range(part * 4, part * 4 + 4):
                nc.tensor.matmul(pq[:], wqk_sb[:, kt, ts(nt, 128)],
                                 xT_sb[:, kt, ts(sq, 512)],
                                 start=(kt == 0), stop=(kt == KT - 1))
            if part == 1:
                nc.vector.tensor_scalar(qkT_sb[:, nt, ts(sq, 512)], pq[:],
                                        bqk_sb[:, nt:nt + 1], None, Alu.add)

        def emit_proj(nt, sh, pool, cast, tag="py"):
            # weight-stationary: yT[n, s] = sum_i wproj[i, n] outT[i, s]
            py = pool.tile([128, SCK], F32, tag=tag)
            for it in range(2):
                for sq in range(2):
                    nc.tensor.matmul(
                        py[:, ts(sq, 512)],
                        wproj_sb[:, it, ts(nt, 128)],
                        outT_sb[:, it, sh * SCK + sq * 512:
                                sh * SCK + sq * 512 + 512],
                        start=(it == 0), stop=(it == 1))
            y_sb = ystage.tile([128, SCK], BF16, tag="y_sb")
            if cast == "dve":
                nc.vector.tensor_copy(y_sb[:], py[:])
            elif cast == "act":
                nc.scalar.copy(y_sb[:], py[:])
            else:
                nc.vector.tensor_copy(y_sb[:, 0:512], py[:, 0:512])
                nc.scalar.copy(y_sb[:, 512:SCK], py[:, 512:SCK])
            nc.sync.dma_start(y_d[nt, :, ts(sh, SCK)], y_sb[:])

        o2s_all = {}
        for pair in range(2):
            ha, hb = 2 * pair, 2 * pair + 1
            qT2 = qkT_sb[:, pair, :]
            kT2 = qkT_sb[:, 2 + pair, :]
            o2s = {}
            for hl in (ha, hb):
                for scH in range(NSC):
                    o2s[(hl, scH)] = o2pool.tile(
                        [HD + 1, SCK], F32, tag=f"o2s_{hl % 2}_{scH}",
                        name=f"o2s_{hl}_{scH}")
            o2s_all.update(o2s)
            for sc4 in range(4):                  # s chunks of 512
                scH, half = sc4 // 2, sc4 % 2
                s0 = sc4 * 512

                po2_a = po2p.tile([HD + 1, 512], F32, tag="po2a")
                po2_b = po2p.tile([HD + 1, 512], F32, tag="po2b")

                def emit_ev(jt, et, stop):
                    nc.tensor.matmul(po2_a[:], vext_sb[:, jt, ha, :],
                                     et[:, 0:512],
                                     start=(jt == 0), stop=stop)
                    nc.tensor.matmul(po2_b[:], vext_sb[:, jt, hb, :],
                                     et[:, 512:1024],
                                     start=(jt == 0), stop=stop)

                def filler(jt):
                    w = (pair, sc4)
                    if w == (0, 0):
                        emit_c(jt, aux)
                    elif w in ((0, 1), (0, 2), (0, 3), (1, 0), (1, 1)):
                        # spread qk-projection remainder; per-window norm
                        chunks = {(0, 1): [(3, 0), (3, 1)],
                                  (0, 2): [(3, 2), (3, 3)],
                                  (0, 3): [(1, 0), (1, 1)],
                                  (1, 0): [(1, 2)],
                                  (1, 1): [(1, 3)]}[w]
                        if jt in (0, 4, 8, 12) and jt // 8 < len(chunks):
                            nt, sq = chunks[jt // 8]
                            b_chunk(nt, sq, (jt // 4) % 2, aux)
                        nrm = {(0, 2): (0, 0), (0, 3): (1, 0),
                               (1, 0): (0, 1), (1, 1): (1, 1)}.get(w)
                        if nrm is not None:
                            if jt == 2:
                                norm_start(o2s_all[nrm], *nrm)
                            elif jt == 3:
                                norm_piece(*nrm, 0)
                            elif jt == 5:
                                norm_piece(*nrm, 1)
                    elif w == (1, 2):
                        if jt == 0:
                            norm_start(o2s[(2, 0)], 2, 0)
                        elif jt == 1:
                            norm_piece(2, 0, 0)
                        elif jt == 2:
                            norm_piece(2, 0, 1)
                        elif jt == 3:
                            norm_start(o2s[(3, 0)], 3, 0)
                        elif jt == 4:
                            norm_piece(3, 0, 0)
                        elif jt == 5:
                            norm_piece(3, 0, 1)
                        elif jt == 9:
                            emit_proj(0, 0, aux, cast="act")
                        elif jt == 12:
                            emit_proj(1, 0, aux, cast="act")
                    elif w == (1, 3):
                        if jt in (1, 4, 7, 10):
                            emit_proj(2 + (jt - 1) // 3, 0, aux,
                                      cast=("act" if jt < 6 else "split"))

                pend = []           # lag-2 EV pipeline
                for jt in range(JT):
                    psc = pscp.tile([128, SCK], F32, tag="psc")
                    nc.tensor.matmul(
                        psc[:, 0:512], kT2[0:64, ts(jt, 128)],
                        qT2[0:64, s0:s0 + 512], start=True, stop=True)
                    nc.tensor.matmul(
                        psc[:, 512:1024], kT2[64:128, ts(jt, 128)],
                        qT2[64:128, s0:s0 + 512], start=True, stop=True)
                    et = dwork.tile([128, SCK], BF16, tag="et")
                    nc.scalar.activation(et[:], psc[:], Act.Exp, scale=0.125)
                    mtsl = mt_sb[:, scH, jt, ts(half, 512)]
                    nc.vector.tensor_tensor(et[:, 0:512], et[:, 0:512],
                                            mtsl, Alu.mult)
                    nc.vector.tensor_tensor(et[:, 512:1024], et[:, 512:1024],
                                            mtsl, Alu.mult)
                    pend.append((jt, et))
                    if len(pend) > 2:
                        pj, pet = pend.pop(0)
                        emit_ev(pj, pet, stop=False)
                    filler(jt)

                for pj, pet in pend:
                    emit_ev(pj, pet, stop=(pj == JT - 1))
                # drain po2 -> o2s staging (DVE; ACT keeps only the exps)
                nc.vector.tensor_copy(o2s[(ha, scH)][:, ts(half, 512)],
                                      po2_a[:])
                nc.vector.tensor_copy(o2s[(hb, scH)][:, ts(half, 512)],
                                      po2_b[:])

                if pair == 0 and sc4 == 0:
                    # v done: aux becomes the phase-B remainder pool
                    aux.release()
                    aux = tc.alloc_tile_pool(name="pb2", bufs=2, space="PSUM")
                elif pair == 1 and sc4 == 1:
                    # B done: aux becomes the phase-E pool (1 buf: 2 banks)
                    aux.release()
                    aux = tc.alloc_tile_pool(name="pe0", bufs=1, space="PSUM")

        # ---- tail: leftover sh0 projections first (their casts land at
        # the head of the DVE queue so the py-tag WARs resolve fast), then
        # the final norms, then the sh1 projections staggered ----
        aux.release()
        po2p.release()
        pscp.release()
        pep = tc.alloc_tile_pool(name="pe1", bufs=1, space="PSUM")
        wt = tc.alloc_tile_pool(name="wt", bufs=1, space="PSUM")
        warm2 = wt.tile([128, 128], F32, tag="w2", bufs=1)

        def wspace(n):
            for _ in range(n):
                nc.tensor.matmul(warm2[:], ones_f32[:], ones_f32[:],
                                 start=True, stop=True,
                                 skip_group_check=True)

        emit_proj(6, 0, pep, cast="dve", tag="py1_0")
        wspace(1)
        emit_proj(7, 0, pep, cast="act", tag="py1_1")
        norm_start(o2s_all[(2, 1)], 2, 1)
        norm_piece(2, 1, 0)
        norm_piece(2, 1, 1)
        norm_start(o2s_all[(3, 1)], 3, 1)
        norm_piece(3, 1, 0)
        norm_piece(3, 1, 1)
        wspace(2)

        pys = {}
        for nt in range(KT):
            py = pep.tile([128, SCK], F32, tag=f"py1_{nt % 3}",
                          name=f"py1_{nt}")
            pys[nt] = py
            nc.tensor.matmul(py[:, 0:512], wproj_sb[:, 0, ts(nt, 128)],
                             outT_sb[:, 0, SCK:SCK + 512],
                             start=True, stop=False)
            nc.tensor.matmul(py[:, 512:SCK], wproj_sb[:, 0, ts(nt, 128)],
                             outT_sb[:, 0, SCK + 512:2 * SCK],
                             start=True, stop=False)
            if nt < 5:
                wspace(2)
            if nt >= 2:
                finish_nt = nt - 2
                pyf = pys[finish_nt]
                for sq in range(2):
                    nc.tensor.matmul(
                        pyf[:, ts(sq, 512)],
                        wproj_sb[:, 1, ts(finish_nt, 128)],
                        outT_sb[:, 1, SCK + sq * 512:SCK + sq * 512 + 512],
                        start=False, stop=True)
                y_sb = ystage.tile([128, SCK], BF16, tag="y_sb")
                nc.vector.tensor_copy(y_sb[:, 0:512], pyf[:, 0:512])
                nc.scalar.copy(y_sb[:, 512:SCK], pyf[:, 512:SCK])
                nc.sync.dma_start(y_d[finish_nt, :, ts(1, SCK)], y_sb[:])
        for nt in range(KT - 2, KT):
            pyf = pys[nt]
            for sq in range(2):
                nc.tensor.matmul(
                    pyf[:, ts(sq, 512)],
                    wproj_sb[:, 1, ts(nt, 128)],
                    outT_sb[:, 1, SCK + sq * 512:SCK + sq * 512 + 512],
                    start=False, stop=True)
            y_sb = ystage.tile([128, SCK], BF16, tag="y_sb")
            nc.vector.tensor_copy(y_sb[:, 0:512], pyf[:, 0:512])
            nc.scalar.copy(y_sb[:, 512:SCK], pyf[:, 512:SCK])
            nc.sync.dma_start(y_d[nt, :, ts(1, SCK)], y_sb[:])

        wt.release()
        pep.release()
        ystage.release()
        nwork.release()
        o2pool.release()
        dwork.release()
        xtp.release()

    nc.compile()
    return nc


def _get_nc():
    global _CACHED_NC
    if _CACHED_NC is None:
        _CACHED_NC = _build_bass()
    return _CACHED_NC


def _prep_core_inputs(x, W_qkv, b_qkv, W_proj, routes_m_T):
    """Host-side shard prep for one (batch b, head-group hg) core."""
    maps = []
    for core in range(NCORES):
        b, hg = core // HG, core % HG
        c0 = hg * DH
        xT = np.ascontiguousarray(x[b].T).astype(bf16)            # (1024, 2048)
        wqk = np.concatenate(
            [W_qkv[:, c0:c0 + DH], W_qkv[:, D + c0:D + c0 + DH],
             W_qkv[:, 2 * D + c0:2 * D + c0 + DH]], axis=1)        # (1024, 768)
        bqk = np.concatenate([b_qkv[c0:c0 + DH], b_qkv[D + c0:D + c0 + DH],
                              b_qkv[2 * D + c0:2 * D + c0 + DH]])
        wproj = W_proj[c0:c0 + DH, :]                              # (256, 1024)
        maps.append({
            "xT": np.ascontiguousarray(xT.reshape(KT, 128, S).transpose(1, 0, 2)),
            "wqk": np.ascontiguousarray(
                wqk.astype(bf16).reshape(KT, 128, 3 * DH).transpose(1, 0, 2)),
            "wproj": np.ascontiguousarray(
                wproj.astype(bf16).reshape(2, 128, D).transpose(1, 0, 2)),
            "mt": routes_m_T,
            "bqk": np.ascontiguousarray(
                bqk.astype(np.float32).reshape(6, 128).T),
        })
    return maps


def kernel(x, W_qkv, b_qkv, W_proj, b_proj, routes):
    x = np.asarray(x, dtype=np.float32)
    W_qkv = np.asarray(W_qkv, dtype=np.float32)
    b_qkv = np.asarray(b_qkv, dtype=np.float32)
    W_proj = np.asarray(W_proj, dtype=np.float32)
    b_proj = np.asarray(b_proj, dtype=np.float32)
    r = np.clip(np.asarray(routes).astype(np.int64), 0, S - 1)

    # multiplicity matrix, uploaded transposed in scH halves:
    # mt[p, scH, jt, sH] = m[scH*1024 + sH, jt*128 + p]
    m = np.zeros((S, S), dtype=np.float32)
    np.add.at(m, (np.arange(S)[:, None].repeat(K, 1).ravel(), r.ravel()), 1.0)
    mT = np.ascontiguousarray(
        m.T.astype(bf16).reshape(JT, 128, NSC, SCK).transpose(1, 2, 0, 3))

    nc = _get_nc()
    in_maps = _prep_core_inputs(x, W_qkv, b_qkv, W_proj, mT)
    res = run_bass_kernel_spmd(nc, in_maps, core_ids=list(range(NCORES)))
    global _LAST_RESULTS
    _LAST_RESULTS = res

    y = np.zeros((B, S, D), dtype=np.float32)
    for core in range(NCORES):
        b = core // HG
        yT = res.results[core]["y"].astype(np.float32)   # (KT, 128, S)
        y[b] += yT.reshape(D, S).T
    y += b_proj[None, None, :]
    return y


# revision 12
# speedup vs baseline: 1.3611x; 1.0656x over previous
"""Trainium2 Bass kernel for CantorGlobalAttention (sparse routed attention).

Strategy: the routes table is shared across batch and heads, so the sparse
gather-attention is reformulated as dense attention using a host-precomputed
route-multiplicity matrix m[s,j] = #{k: routes[s,k] = j}:

    out[s] = (sum_j m[s,j] exp(SC[s,j]) v[j]) / (sum_j m[s,j] exp(SC[s,j]))
    SC = q @ k^T / sqrt(HD)

Everything runs in a transposed layout (feature dim on partitions) so no
on-device transposes are needed anywhere:
    qkT[n,s]  = (W_qk^T x^T)              (W stationary)
    SCT[j,s]  = k^T(j-tile)^T q^T         (K=64 matmul)
    ET        = mT * exp(0.125 * SCT)     (ACT exp + DVE mult, bf16)
    o2T[c,s]  = [v|1]^T @ ET              (ones col -> softmax denom Z)
    outT      = o2T[0:64] * (1/Z)         (Z bcast on GPSIMD, recip on DVE)
    y[s,n]    = outT^T @ W_proj(rows)     (per-core partial, bf16 out)

Phase D runs a lag-1 software pipeline: the EV matmuls for j-tile jt-1 are
emitted after the SCT matmuls for jt, so the PE never waits on the ACT
exp / DVE multiply chain. B (remaining qk proj), C (v proj) and E (output
proj, first s-half) are folded into phase-D windows as PE filler.

Sharding: 8 cores = 2 batches x 4 head-groups (4 heads each). Host sums the
4 per-batch partials and adds b_proj.
"""

import numpy as np
import ml_dtypes
from contextlib import ExitStack

import concourse.bacc as bacc
import concourse.mybir as mybir
import concourse.tile as tile
from concourse.bass import ts
from concourse.bass_utils import run_bass_kernel_spmd

bf16 = ml_dtypes.bfloat16
F32 = mybir.dt.float32
BF16 = mybir.dt.bfloat16
Alu = mybir.AluOpType
Act = mybir.ActivationFunctionType

B, S, D = 2, 2048, 1024
H, HD, K = 16, 64, 64
NCORES = 8
HG = 4            # head-groups (cores per batch)
NH = H // HG      # heads per core = 4
DH = NH * HD      # feature cols per core for q/k/v = 256
ST = S // 128     # 16 s-tiles
JT = S // 128     # 16 j-tiles
KT = D // 128     # 8 contraction tiles for the projections
SCK = 1024        # s-chunk for phase E / o2s staging
NSC = S // SCK    # 2

_CACHED_NC = None
_LAST_RESULTS = None


def _build_bass():
    nc = bacc.Bacc("TRN2", target_bir_lowering=False, debug=False)

    xT_d = nc.dram_tensor("xT", [128, KT, S], BF16, kind="ExternalInput")
    wqk_d = nc.dram_tensor("wqk", [128, KT, 3 * DH], BF16, kind="ExternalInput")
    wproj_d = nc.dram_tensor("wproj", [128, 2, D], BF16, kind="ExternalInput")
    mt_d = nc.dram_tensor("mt", [128, NSC, JT, SCK], BF16, kind="ExternalInput")
    bqk_d = nc.dram_tensor("bqk", [128, 6], F32, kind="ExternalInput")
    y_d = nc.dram_tensor("y", [KT, 128, S], BF16, kind="ExternalOutput")

    with tile.TileContext(nc) as tc, ExitStack() as ctx:
        cp = ctx.enter_context(tc.tile_pool(name="consts", bufs=1))

        wqk_sb = cp.tile([128, KT, 3 * DH], BF16)
        wproj_sb = cp.tile([128, 2, D], BF16)
        mt_sb = cp.tile([128, NSC, JT, SCK], BF16)
        bqk_sb = cp.tile([128, 6], F32)
        ones_f32 = cp.tile([128, 128], F32)
        qkT_sb = cp.tile([128, 4, S], BF16)  # nt 0,1 = qT; 2,3 = kT
        vext_sb = cp.tile([128, ST, NH, HD + 1], BF16)
        outT_sb = cp.tile([128, 2, S], BF16)     # proj lhsT layout

        xtp = tc.alloc_tile_pool(name="xtp", bufs=1)
        xT_sb = xtp.tile([128, KT, S], BF16)

        # loads, in first-use order; wqk/xT interleaved per kt so the first
        # phase-B wave starts as early as possible; mt comes as scH halves
        # so phase D can start as soon as the first scH0 j-tiles land
        for kt in range(KT):
            nc.sync.dma_start(wqk_sb[:, kt, :], wqk_d[:, kt, :])
            nc.sync.dma_start(xT_sb[:, kt, :], xT_d[:, kt, :])
        nc.sync.dma_start(bqk_sb[:], bqk_d[:])
        for jt in range(JT):
            nc.sync.dma_start(mt_sb[:, 0, jt, :], mt_d[:, 0, jt, :])
        nc.sync.dma_start(wproj_sb[:], wproj_d[:])
        for jt in range(JT):
            nc.sync.dma_start(mt_sb[:, 1, jt, :], mt_d[:, 1, jt, :])

        nc.vector.memset(ones_f32[:], 1.0)
        # ones column FIRST so the softmax denom Z lands on partition 0 of
        # po2/o2s (gpsimd partition_broadcast can only source partition 0)
        nc.vector.memset(vext_sb[:, :, :, 0:1], 1.0)
        # touch Exp now so the ACT table set loads during the DMA window,
        # not at the first real exp
        tblwarm = cp.tile([1, 8], F32)
        nc.scalar.activation(tblwarm[:], ones_f32[0:1, 0:8], Act.Exp)

        # ---- PE warmup during the initial DMA window (HAM clock gate) ----
        pwarm = tc.alloc_tile_pool(name="pwarm", bufs=1, space="PSUM")
        warm = pwarm.tile([128, 128], F32, tag="warm", bufs=1)
        for _ in range(24):
            nc.tensor.matmul(warm[:], ones_f32[:], ones_f32[:],
                             start=True, stop=True, skip_group_check=True)
        pwarm.release()

        def emit_copy(nt, sq, pq, eng):
            if eng == 0:
                nc.vector.tensor_scalar(
                    qkT_sb[:, nt, ts(sq, 512)], pq[:],
                    bqk_sb[:, nt:nt + 1], None, Alu.add)
            else:
                nc.scalar.activation(
                    qkT_sb[:, nt, ts(sq, 512)], pq[:],
                    Act.Identity, bias=bqk_sb[:, nt:nt + 1])

        # ---- phase B (q,k of pair 0) as kt-waves: matmuls start as each xT
        # k-tile DMA lands; one LDWEIGHTS covers four matmuls ----
        pbp = tc.alloc_tile_pool(name="pbp", bufs=1, space="PSUM")
        b02 = [(nt, sq) for nt in (0, 2) for sq in range(S // 512)]
        pqs = {g: pbp.tile([128, 512], F32, tag=f"pq_{g[0]}_{g[1]}",
                           name=f"pq_{g[0]}_{g[1]}") for g in b02}
        for kt in range(KT):
            for nt in (0, 2):
                for sq in range(S // 512):
                    nc.tensor.matmul(
                        pqs[(nt, sq)][:],
                        wqk_sb[:, kt, ts(nt, 128)],
                        xT_sb[:, kt, ts(sq, 512)],
                        start=(kt == 0), stop=(kt == KT - 1),
                    )
        for sq in range(S // 512):           # sq0 copies first: D needs them
            for i, nt in enumerate((0, 2)):
                eng = 0 if sq == 0 else (sq + i) % 2
                emit_copy(nt, sq, pqs[(nt, sq)], eng)
        pbp.release()

        def emit_c(st, pool):
            """v[st] in normal layout (x-stationary) -> vext; copy on DVE/ACT."""
            pv = pool.tile([128, DH], F32, tag="pv")
            for kt in range(KT):
                nc.tensor.matmul(
                    pv[:], xT_sb[:, kt, ts(st, 128)],
                    wqk_sb[:, kt, 4 * 128:6 * 128],
                    start=(kt == 0), stop=(kt == KT - 1))
            dst = vext_sb[:, st, :, 1:HD + 1]
            src = pv[:].rearrange("p (h d) -> p h d", d=HD)
            nc.vector.tensor_copy(dst, src)

        def emit_b_pair(nt, sp, pool):
            # two sq chunks, kt-major: one LDWEIGHTS serves two matmuls
            sqs = (2 * sp, 2 * sp + 1)
            pq2 = {sq: pool.tile([128, 512], F32, tag=f"pq2_{sq % 2}",
                                 name=f"pq2_{nt}_{sq}") for sq in sqs}
            for kt in range(KT):
                for sq in sqs:
                    nc.tensor.matmul(
                        pq2[sq][:],
                        wqk_sb[:, kt, ts(nt, 128)],
                        xT_sb[:, kt, ts(sq, 512)],
                        start=(kt == 0), stop=(kt == KT - 1),
                    )
            for i, sq in enumerate(sqs):
                emit_copy(nt, sq, pq2[sq], i)

        # ---- phase D: attention, lag-1 pipelined ----
        dwork = tc.alloc_tile_pool(name="dwork", bufs=4)
        o2pool = tc.alloc_tile_pool(name="o2pool", bufs=1)
        nwork = tc.alloc_tile_pool(name="nwork", bufs=2)

        pscp = tc.alloc_tile_pool(name="pscp", bufs=2, space="PSUM")
        po2p = tc.alloc_tile_pool(name="po2p", bufs=1, space="PSUM")
        aux = tc.alloc_tile_pool(name="pvp", bufs=2, space="PSUM")

        ystage = tc.alloc_tile_pool(name="ystage", bufs=3)

        norm_state = {}

        def norm_start(o2s_t, hl, scH):
            # Z (partition 0) broadcast on GPSIMD; recip/scale follow as
            # half-width DVE pieces spread over later slots
            zb = nwork.tile([HD + 1, SCK], F32, tag="zb", bufs=2)
            nc.gpsimd.partition_broadcast(zb[:], o2s_t[0:1, :])
            rz = nwork.tile([HD + 1, SCK], F32, tag="rz", bufs=2)
            stage = nwork.tile([HD + 1, SCK], BF16, tag="stage", bufs=2)
            norm_state[(hl, scH)] = (o2s_t, zb, rz, stage)

        def norm_piece(hl, scH, half):
            o2s_t, zb, rz, stage = norm_state[(hl, scH)]
            sl = ts(half, 512)
            nc.vector.reciprocal_approx_fast(out=rz[:, sl], in_=zb[:, sl])
            nc.vector.tensor_tensor(stage[:, sl], o2s_t[:, sl], rz[:, sl],
                                    Alu.mult)
            if half == 1:
                p0 = (hl % 2) * 64
                nc.sync.dma_start(
                    outT_sb[p0:p0 + 64, hl // 2, ts(scH, SCK)],
                    stage[1:HD + 1, :])

        bstate = {}

        def b_chunk(nt, sq, part, pool):
            # half of a (nt, sq) qk-projection chunk: 4 kt steps; the second
            # half finishes the accumulation and drains with the bias add
            if part == 0:
                bstate[(nt, sq)] = pool.tile([128, 512], F32, tag="pq",
                                             name=f"pq_{nt}_{sq}")
            pq = bstate[(nt, sq)]
            for kt in range(part * 4, part * 4 + 4):
                nc.tensor.matmul(pq[:], wqk_sb[:, kt, ts(nt, 128)],
                                 xT_sb[:, kt, ts(sq, 512)],
                                 start=(kt == 0), stop=(kt == KT - 1))
            if part == 1:
                nc.vector.tensor_scalar(qkT_sb[:, nt, ts(sq, 512)], pq[:],
                                        bqk_sb[:, nt:nt + 1], None, Alu.add)

        def emit_proj(nt, sh, pool, cast, tag="py"):
            # weight-stationary: yT[n, s] = sum_i wproj[i, n] outT[i, s]
            py = pool.tile([128, SCK], F32, tag=tag)
            for it in range(2):
                for sq in range(2):
                    nc.tensor.matmul(
                        py[:, ts(sq, 512)],
                        wproj_sb[:, it, ts(nt, 128)],
                        outT_sb[:, it, sh * SCK + sq * 512:
                                sh * SCK + sq * 512 + 512],
                        start=(it == 0), stop=(it == 1))
            y_sb = ystage.tile([128, SCK], BF16, tag="y_sb")
            if cast == "dve":
                nc.vector.tensor_copy(y_sb[:], py[:])
            elif cast == "act":
                nc.scalar.copy(y_sb[:], py[:])
            else:
                nc.vector.tensor_copy(y_sb[:, 0:512], py[:, 0:512])
                nc.scalar.copy(y_sb[:, 512:SCK], py[:, 512:SCK])
            nc.sync.dma_start(y_d[nt, :, ts(sh, SCK)], y_sb[:])

        o2s_all = {}
        for pair in range(2):
            ha, hb = 2 * pair, 2 * pair + 1
            qT2 = qkT_sb[:, pair, :]
            kT2 = qkT_sb[:, 2 + pair, :]
            o2s = {}
            for hl in (ha, hb):
                for scH in range(NSC):
                    o2s[(hl, scH)] = o2pool.tile(
                        [HD + 1, SCK], F32, tag=f"o2s_{hl % 2}_{scH}",
                        name=f"o2s_{hl}_{scH}")
            o2s_all.update(o2s)
            for sc4 in range(4):                  # s chunks of 512
                scH, half = sc4 // 2, sc4 % 2
                s0 = sc4 * 512

                po2_a = po2p.tile([HD + 1, 512], F32, tag="po2a")
                po2_b = po2p.tile([HD + 1, 512], F32, tag="po2b")

                def emit_ev(jt, et, stop):
                    nc.tensor.matmul(po2_a[:], vext_sb[:, jt, ha, :],
                                     et[:, 0:512],
                                     start=(jt == 0), stop=stop)
                    nc.tensor.matmul(po2_b[:], vext_sb[:, jt, hb, :],
                                     et[:, 512:1024],
                                     start=(jt == 0), stop=stop)

                def filler(jt):
                    w = (pair, sc4)
                    if w == (0, 0):
                        emit_c(jt, aux)
                    elif w in ((0, 1), (0, 2), (0, 3), (1, 0), (1, 1)):
                        # spread qk-projection remainder; per-window norm
                        chunks = {(0, 1): [(3, 0), (3, 1)],
                                  (0, 2): [(3, 2), (3, 3)],
                                  (0, 3): [(1, 0), (1, 1)],
                                  (1, 0): [(1, 2)],
                                  (1, 1): [(1, 3)]}[w]
                        if jt in (0, 4, 8, 12) and jt // 8 < len(chunks):
                            nt, sq = chunks[jt // 8]
                            b_chunk(nt, sq, (jt // 4) % 2, aux)
                        nrm = {(0, 2): (0, 0), (0, 3): (1, 0),
                               (1, 0): (0, 1), (1, 1): (1, 1)}.get(w)
                        if nrm is not None:
                            if jt == 2:
                                norm_start(o2s_all[nrm], *nrm)
                            elif jt == 3:
                                norm_piece(*nrm, 0)
                            elif jt == 5:
                                norm_piece(*nrm, 1)
                    elif w == (1, 2):
                        if jt == 0:
                            norm_start(o2s[(2, 0)], 2, 0)
                        elif jt == 1:
                            norm_piece(2, 0, 0)
                        elif jt == 2:
                            norm_piece(2, 0, 1)
                        elif jt == 3:
                            norm_start(o2s[(3, 0)], 3, 0)
                        elif jt == 4:
                            norm_piece(3, 0, 0)
                        elif jt == 5:
                            norm_piece(3, 0, 1)
                        elif jt == 9:
                            emit_proj(0, 0, aux, cast="act")
                        elif jt == 12:
                            emit_proj(1, 0, aux, cast="act")
                    elif w == (1, 3):
                        if jt in (1, 4, 7, 10):
                            emit_proj(2 + (jt - 1) // 3, 0, aux,
                                      cast=("act" if jt < 6 else "split"))

                pend = []           # lag-2 EV pipeline
                for jt in range(JT):
                    psc = pscp.tile([128, SCK], F32, tag="psc")
                    nc.tensor.matmul(
                        psc[:, 0:512], kT2[0:64, ts(jt, 128)],
                        qT2[0:64, s0:s0 + 512], start=True, stop=True)
                    nc.tensor.matmul(
                        psc[:, 512:1024], kT2[64:128, ts(jt, 128)],
                        qT2[64:128, s0:s0 + 512], start=True, stop=True)
                    et = dwork.tile([128, SCK], BF16, tag="et")
                    nc.scalar.activation(et[:], psc[:], Act.Exp, scale=0.125)
                    # one fused multiply over both head-halves: the mt slice is
                    # applied twice via a 0-stride broadcast dim
                    mtsl = mt_sb[:, scH:scH + 1, jt,
                                 ts(half, 512)].broadcast_to([128, 2, 512])
                    etv = et[:].rearrange("p (a b) -> p a b", a=2)
                    nc.vector.tensor_tensor(etv, etv, mtsl, Alu.mult)
                    pend.append((jt, et))
                    if len(pend) > 2:
                        pj, pet = pend.pop(0)
                        emit_ev(pj, pet, stop=False)
                    filler(jt)

                for pj, pet in pend:
                    emit_ev(pj, pet, stop=(pj == JT - 1))
                # drain po2 -> o2s staging (DVE; ACT keeps only the exps)
                nc.vector.tensor_copy(o2s[(ha, scH)][:, ts(half, 512)],
                                      po2_a[:])
                nc.vector.tensor_copy(o2s[(hb, scH)][:, ts(half, 512)],
                                      po2_b[:])

                if pair == 0 and sc4 == 0:
                    # v done: aux becomes the phase-B remainder pool
                    aux.release()
                    aux = tc.alloc_tile_pool(name="pb2", bufs=2, space="PSUM")
                elif pair == 1 and sc4 == 1:
                    # B done: aux becomes the phase-E pool (1 buf: 2 banks)
                    aux.release()
                    aux = tc.alloc_tile_pool(name="pe0", bufs=1, space="PSUM")

        # ---- tail: leftover sh0 projections first (their casts land at
        # the head of the DVE queue so the py-tag WARs resolve fast), then
        # the final norms, then the sh1 projections staggered ----
        aux.release()
        po2p.release()
        pscp.release()
        pep = tc.alloc_tile_pool(name="pe1", bufs=1, space="PSUM")
        wt = tc.alloc_tile_pool(name="wt", bufs=1, space="PSUM")
        warm2 = wt.tile([128, 128], F32, tag="w2", bufs=1)

        def wspace(n):
            for _ in range(n):
                nc.tensor.matmul(warm2[:], ones_f32[:], ones_f32[:],
                                 start=True, stop=True,
                                 skip_group_check=True)

        emit_proj(6, 0, pep, cast="dve", tag="py1_0")
        wspace(1)
        emit_proj(7, 0, pep, cast="act", tag="py1_1")
        norm_start(o2s_all[(2, 1)], 2, 1)
        norm_piece(2, 1, 0)
        norm_piece(2, 1, 1)
        norm_start(o2s_all[(3, 1)], 3, 1)
        norm_piece(3, 1, 0)
        norm_piece(3, 1, 1)
        wspace(2)

        pys = {}
        for nt in range(KT):
            py = pep.tile([128, SCK], F32, tag=f"py1_{nt % 3}",
                          name=f"py1_{nt}")
            pys[nt] = py
            nc.tensor.matmul(py[:, 0:512], wproj_sb[:, 0, ts(nt, 128)],
                             outT_sb[:, 0, SCK:SCK + 512],
                             start=True, stop=False)
            nc.tensor.matmul(py[:, 512:SCK], wproj_sb[:, 0, ts(nt, 128)],
                             outT_sb[:, 0, SCK + 512:2 * SCK],
                             start=True, stop=False)
            if nt < 5:
                wspace(2)
            if nt >= 2:
                finish_nt = nt - 2
                pyf = pys[finish_nt]
                for sq in range(2):
                    nc.tensor.matmul(
                        pyf[:, ts(sq, 512)],
                        wproj_sb[:, 1, ts(finish_nt, 128)],
                        outT_sb[:, 1, SCK + sq * 512:SCK + sq * 512 + 512],
                        start=False, stop=True)
                y_sb = ystage.tile([128, SCK], BF16, tag="y_sb")
                nc.vector.tensor_copy(y_sb[:, 0:512], pyf[:, 0:512])
                nc.scalar.copy(y_sb[:, 512:SCK], pyf[:, 512:SCK])
                nc.sync.dma_start(y_d[finish_nt, :, ts(1, SCK)], y_sb[:])
        for nt in range(KT - 2, KT):
            pyf = pys[nt]
            for sq in range(2):
                nc.tensor.matmul(
                    pyf[:, ts(sq, 512)],
                    wproj_sb[:, 1, ts(nt, 128)],
                    outT_sb[:, 1, SCK + sq * 512:SCK + sq * 512 + 512],
                    start=False, stop=True)
            y_sb = ystage.tile([128, SCK], BF16, tag="y_sb")
            nc.vector.tensor_copy(y_sb[:, 0:512], pyf[:, 0:512])
            nc.scalar.copy(y_sb[:, 512:SCK], pyf[:, 512:SCK])
            nc.sync.dma_start(y_d[nt, :, ts(1, SCK)], y_sb[:])

        wt.release()
        pep.release()
        ystage.release()
        nwork.release()
        o2pool.release()
        dwork.release()
        xtp.release()

    nc.compile()
    return nc


def _get_nc():
    global _CACHED_NC
    if _CACHED_NC is None:
        _CACHED_NC = _build_bass()
    return _CACHED_NC


def _prep_core_inputs(x, W_qkv, b_qkv, W_proj, routes_m_T):
    """Host-side shard prep for one (batch b, head-group hg) core."""
    maps = []
    for core in range(NCORES):
        b, hg = core // HG, core % HG
        c0 = hg * DH
        xT = np.ascontiguousarray(x[b].T).astype(bf16)            # (1024, 2048)
        wqk = np.concatenate(
            [W_qkv[:, c0:c0 + DH], W_qkv[:, D + c0:D + c0 + DH],
             W_qkv[:, 2 * D + c0:2 * D + c0 + DH]], axis=1)        # (1024, 768)
        bqk = np.concatenate([b_qkv[c0:c0 + DH], b_qkv[D + c0:D + c0 + DH],
                              b_qkv[2 * D + c0:2 * D + c0 + DH]])
        wproj = W_proj[c0:c0 + DH, :]                              # (256, 1024)
        maps.append({
            "xT": np.ascontiguousarray(xT.reshape(KT, 128, S).transpose(1, 0, 2)),
            "wqk": np.ascontiguousarray(
                wqk.astype(bf16).reshape(KT, 128, 3 * DH).transpose(1, 0, 2)),
            "wproj": np.ascontiguousarray(
                wproj.astype(bf16).reshape(2, 128, D).transpose(1, 0, 2)),
            "mt": routes_m_T,
            "bqk": np.ascontiguousarray(
                bqk.astype(np.float32).reshape(6, 128).T),
        })
    return maps


def kernel(x, W_qkv, b_qkv, W_proj, b_proj, routes):
    x = np.asarray(x, dtype=np.float32)
    W_qkv = np.asarray(W_qkv, dtype=np.float32)
    b_qkv = np.asarray(b_qkv, dtype=np.float32)
    W_proj = np.asarray(W_proj, dtype=np.float32)
    b_proj = np.asarray(b_proj, dtype=np.float32)
    r = np.clip(np.asarray(routes).astype(np.int64), 0, S - 1)

    # multiplicity matrix, uploaded transposed in scH halves:
    # mt[p, scH, jt, sH] = m[scH*1024 + sH, jt*128 + p]
    m = np.zeros((S, S), dtype=np.float32)
    np.add.at(m, (np.arange(S)[:, None].repeat(K, 1).ravel(), r.ravel()), 1.0)
    mT = np.ascontiguousarray(
        m.T.astype(bf16).reshape(JT, 128, NSC, SCK).transpose(1, 2, 0, 3))

    nc = _get_nc()
    in_maps = _prep_core_inputs(x, W_qkv, b_qkv, W_proj, mT)
    res = run_bass_kernel_spmd(nc, in_maps, core_ids=list(range(NCORES)))
    global _LAST_RESULTS
    _LAST_RESULTS = res

    y = np.zeros((B, S, D), dtype=np.float32)
    for core in range(NCORES):
        b = core // HG
        yT = res.results[core]["y"].astype(np.float32)   # (KT, 128, S)
        y[b] += yT.reshape(D, S).T
    y += b_proj[None, None, :]
    return y
